# revision 32
# baseline (speedup 1.0000x reference)
"""Trainium2 Bass kernel for nn_Covid19InfectionsPredictModel.

Model: rate = relu(inputs @ a)  [T];  a strictly-sequential 20-tap linear
recurrence s_t = rate_t * dot(s_{t-20..t-1}, rev_head) seeded by a unit
impulse, and the observation FIR out_t = dot(s_{t-20..t-1}, diff).

Key structural fact (pure IEEE-754 float32, no approximation): the weight
rev_head[0] is exactly 0 and diff[j] > 0 for all j. The recurrence explodes
(growth ~14x/step for the given parameter scale), hits +inf, and then
0*inf => NaN poisons the window. Once the 20-value state window is ALL-NaN
at some step t*, every later s_t and out_t is NaN *regardless of rate*
(NaN*w + ... = NaN, and rate*NaN = NaN even for rate == 0). So the exact
full output is determined by the first ~60 steps plus a runtime-verified
all-NaN certificate.

Strategy:
  1. One small NEFF computes the first K=96 steps exactly on-device:
     rate via TensorE matmuls (the host passes the 96-row input slice
     pre-transposed - a pure layout change), per-step pre-scaled weights
     W[t] = rate_t * rev_head as a TensorE outer product flattened
     through a DRAM bounce, the sequential recurrence as ONE fused DVE
     scalar_tensor_tensor (accum_out) per step, the output FIR as a
     20-tap scalar_tensor_tensor chain, plus the NaN tail out[96:] as a
     single DRAM->DRAM broadcast DMA and the final 20-value state window.
  2. Host checks the returned state window. All-NaN (the certified,
     overwhelmingly common case) => done. Otherwise a fallback chunk
     NEFF (1024 steps per launch, same math) is compiled lazily and
     looped over the remaining sequence, so the kernel is exactly
     correct for ANY inputs, just slower in the never-taken branch.

The recurrence is strictly sequential and tiny per step; there is no
batch/scenario dimension in this problem instance, so the 8 cores run the
program SPMD-replicated (core 0's output is gathered) - intra-sequence
sharding has nothing to win (see sharding hint). The sequential step is
latency-bound at one small DVE op (~165ns); step-pairing rewrites were
rejected because they provably shift the inf->NaN onset by one position.
"""

from contextlib import ExitStack

import numpy as np

T_FULL = 65536
NW = 64
L = 21
WIN = 20  # recurrence window (L-1 taps)
K_HAPPY = 96
K_CHUNK = 1024
NAN_COLS = 2045  # (T_FULL - K_HAPPY) == 32 * 2045

_cache = {}


def _weights_from_h(h):
    """rev_head (wvec) and diff (dvec) exactly as the reference computes them."""
    h0 = np.float32(np.asarray(h).reshape(-1)[0])
    t = np.arange(L, dtype=np.float32)
    surv = np.exp(-t / h0).astype(np.float32)
    surv = ((surv - surv[-1]) / (np.float32(1.0) - surv[-1])).astype(np.float32)
    rev = surv[::-1].copy()  # reverse_surv, rev[20] == 1, rev[0] == 0
    wvec = rev[:WIN].copy()  # rev_head, wvec[0] == 0 exactly
    dvec = (rev[1:] - rev[:-1]).copy()  # diff, all > 0
    return wvec, dvec


def _build(K, full_out, dvec):
    """Emit + compile the K-step recurrence NEFF. Returns the Bacc object.

    dvec (the observation FIR taps, derived from the input h) is baked into
    the NEFF as immediate scalars; the cache key includes its bytes.
    """
    import concourse.bacc as bacc
    import concourse.bass as bass
    import concourse.tile as tile
    from concourse import mybir

    f32 = mybir.dt.float32
    mult = mybir.AluOpType.mult
    add = mybir.AluOpType.add

    nc = bacc.Bacc("TRN2", num_devices=1, debug=False)
    # xTa = [inputs[t0:t0+K].T | a] : one DMA covers both matmul operands.
    xTa = nc.dram_tensor("xTa", [NW, K + 1], f32, kind="ExternalInput").ap()
    # aux = [wvec(20) | dvec(20) | state0(20)] : one DMA lands the FIR taps,
    # wvec, and the s-buffer seed.
    aux = nc.dram_tensor("aux", [3 * WIN], f32, kind="ExternalInput").ap()
    dv = nc.dram_tensor("dv", [WIN], f32, kind="ExternalInput").ap()
    if full_out:
        nansrc = nc.dram_tensor("nansrc", [NAN_COLS], f32, kind="ExternalInput").ap()
        out = nc.dram_tensor("out", [T_FULL], f32, kind="ExternalOutput").ap()
    else:
        out = nc.dram_tensor("out", [K], f32, kind="ExternalOutput").ap()
    st_out = nc.dram_tensor("st_out", [WIN], f32, kind="ExternalOutput").ap()

    with tile.TileContext(nc) as tc, ExitStack() as ctx:
        pool = ctx.enter_context(tc.tile_pool(name="p", bufs=1))
        psp = ctx.enter_context(tc.tile_pool(name="ps", bufs=1, space="PSUM"))

        xTa_sb = pool.tile([NW, K + 1], f32)
        nc.sync.dma_start(out=xTa_sb, in_=xTa)
        # aux_sb[0, 0:20]=wvec, [20:40]=dvec, [40:60]=state0 = s_buf[0:20].
        aux_sb = pool.tile([1, 2 * WIN + K + WIN], f32)
        nc.sync.dma_start(out=aux_sb[:, : 3 * WIN], in_=aux.rearrange("(o k) -> o k", o=1))
        w_row = aux_sb[:, 0:WIN]
        dv_row = aux_sb[:, WIN : 2 * WIN]
        s_buf = aux_sb[:, 2 * WIN : 2 * WIN + K + WIN]
        d_col = pool.tile([WIN, 1], f32)
        nc.gpsimd.dma_start(out=d_col, in_=dv.rearrange("(k o) -> k o", o=1))

        # rate_row[0, t] = relu(inputs[t] @ a), flat on partition 0 (relu as a
        # DVE max against 0.0 straight out of PSUM - no ScalarE on this path).
        rate_row = pool.tile([1, K], f32)
        for m in range(0, K, 512):
            n = min(512, K - m)
            r_ps = psp.tile([1, 512], f32, tag="r_ps")
            nc.tensor.matmul(
                r_ps[:, :n],
                lhsT=xTa_sb[:, K : K + 1],
                rhs=xTa_sb[:, m : m + n],
                start=True,
                stop=True,
            )
            nc.vector.tensor_scalar_max(rate_row[:, m : m + n], r_ps[:, :n], 0.0)

        # Pre-scaled per-step weights W[t, j] = rate_t * wvec[j]: TensorE outer
        # product (blocked to the 128-partition PSUM limit), flattened t-major
        # by one SBUF->SBUF DMA per block so each step reads a contiguous
        # 20-element slice on partition 0.
        wf = pool.tile([1, K * WIN], f32)
        for c in range(0, K, 128):
            blk = min(128, K - c)
            w_ps = psp.tile([128, WIN], f32, tag="w_ps")
            nc.tensor.matmul(
                w_ps[:blk], lhsT=rate_row[:, c : c + blk], rhs=w_row, start=True, stop=True
            )
            w2d = pool.tile([128, WIN], f32, tag="w2d")
            nc.scalar.activation(w2d[:blk], w_ps[:blk], mybir.ActivationFunctionType.Copy)
            nc.sync.dma_start(out=wf[:, c * WIN : (c + blk) * WIN], in_=w2d[:blk])

        # The sequential chain: one fused multiply+accumulate-reduce per step.
        # First S0 steps read rate_t as a pointer scalar (available ~2.5us
        # before the flattened wf buffer), computing (wvec*rate_t)*s — bitwise
        # identical to the wf form fl(fl(w*r)*s) — so the chain starts as soon
        # as the rate row exists; the rest use the cheaper immediate form.
        S0 = min(16, K)
        junk = pool.tile([1, WIN], f32)
        seq_insts = []
        for t in range(S0):
            i = nc.vector.scalar_tensor_tensor(
                out=junk,
                in0=w_row,
                scalar=rate_row[:, t : t + 1],
                in1=s_buf[:, t : t + WIN],
                op0=mult,
                op1=mult,
                accum_out=s_buf[:, WIN + t : WIN + t + 1],
            )
            seq_insts.append(i)
        for t in range(S0, K):
            o = t * WIN
            i = nc.vector.scalar_tensor_tensor(
                out=junk,
                in0=s_buf[:, t : t + WIN],
                scalar=1.0,
                in1=wf[:, o : o + WIN],
                op0=mult,
                op1=mult,
                accum_out=s_buf[:, WIN + t : WIN + t + 1],
            )
            seq_insts.append(i)

        nc.gpsimd.dma_start(
            out=st_out.rearrange("(o k) -> o k", o=1), in_=s_buf[:, K : K + WIN]
        )

        # Observation FIR out[t] = sum_j s[t+j] * diff[j] as a TensorE matvec
        # over 20 shifted copies of s (one SBUF->SBUF overlapping-window DMA
        # per chunk). Two chunks: the first only needs s[0:h+19], so its whole
        # chain (DMA -> PE -> ScalarE copy -> DMA out) overlaps the DVE
        # sequential loop's back half. Nothing here touches the DVE.
        # PE chunks need their last s value 19 steps before seq end, so both
        # overlap the sequential loop entirely; the final TAILN outputs are
        # per-output fused accum ops on the otherwise-idle GPSIMD, each ready
        # right after the seq step that feeds it.
        TAILN = 19
        kpe = K - TAILN  # 77
        for f0, ln in ((0, (kpe + 1) // 2), ((kpe + 1) // 2, kpe - (kpe + 1) // 2)):
            # Bounce this chunk's s-slice through DRAM: the SBUF-side write
            # uses a plain range-tracked AP (so it waits for exactly the seq
            # steps it needs), and the overlapping-window read is from DRAM
            # where whole-tensor dependency tracking orders it correctly.
            sscr = nc.dram_tensor(f"sscr{f0}", [ln + WIN - 1], f32, kind="Internal").ap()
            nc.sync.dma_start(
                out=sscr.rearrange("(o k) -> o k", o=1),
                in_=s_buf[:, f0 : f0 + ln + WIN - 1],
            )
            s_sh = pool.tile([WIN, ln], f32, name=f"ssh{f0}")
            src = bass.AP(tensor=sscr.tensor, offset=sscr.offset, ap=[[1, WIN], [1, ln]])
            nc.sync.dma_start(out=s_sh, in_=src)
            o_ps = psp.tile([1, 512], f32, tag=f"ops{f0}")
            nc.tensor.matmul(o_ps[:, :ln], lhsT=d_col, rhs=s_sh, start=True, stop=True)
            o_sb = pool.tile([1, ln], f32, name=f"osb{f0}")
            nc.scalar.activation(o_sb, o_ps[:, :ln], mybir.ActivationFunctionType.Copy)
            nc.sync.dma_start(
                out=out[f0 : f0 + ln].rearrange("(o k) -> o k", o=1), in_=o_sb
            )

        out_c = pool.tile([1, TAILN], f32)
        junk2 = pool.tile([1, WIN], f32)
        for i in range(TAILN):
            t = kpe + i
            nc.vector.scalar_tensor_tensor(
                out=junk2,
                in0=dv_row,
                scalar=1.0,
                in1=s_buf[:, t : t + WIN],
                op0=mult,
                op1=mult,
                accum_out=out_c[:, i : i + 1],
            )
        nc.sync.dma_start(
            out=out[kpe:K].rearrange("(o k) -> o k", o=1), in_=out_c
        )

        if full_out:
            # NaN tail: one DRAM->DRAM broadcast DMA on the GPSIMD queues,
            # no engine time and no Sync-sequencer issue slot.
            tail_dst = out[K:T_FULL].rearrange("(r c) -> r c", c=NAN_COLS)
            tail_src = bass.AP(
                tensor=nansrc.tensor, offset=nansrc.offset, ap=[[0, 32], [1, NAN_COLS]]
            )
            nc.gpsimd.dma_start(out=tail_dst, in_=tail_src)

    nc.compile()
    return nc


def _get_neff(K, full_out, dvec):
    key = (K, full_out, dvec.tobytes())
    if key not in _cache:
        _cache[key] = _build(K, full_out, dvec)
    return _cache[key]


def _run(nc, feeds, trace=False):
    from concourse import bass_utils

    in_maps = [dict(feeds) for _ in range(8)]
    res = bass_utils.run_bass_kernel_spmd(nc, in_maps, core_ids=list(range(8)), trace=trace)
    return res


def _feeds(inputs, a, wvec, dvec, state, t0, K, full_out):
    xTa = np.empty((NW, K + 1), dtype=np.float32)
    xTa[:, :K] = inputs[t0 : t0 + K].T
    xTa[:, K] = a
    aux = np.concatenate([wvec, dvec, state]).astype(np.float32)
    f = {"xTa": xTa, "aux": aux, "dv": dvec}
    if full_out:
        f["nansrc"] = np.full(NAN_COLS, np.nan, dtype=np.float32)
    return f


def kernel(inputs, a, h):
    inputs = np.ascontiguousarray(np.asarray(inputs, dtype=np.float32))
    a = np.ascontiguousarray(np.asarray(a, dtype=np.float32))
    wvec, dvec = _weights_from_h(h)
    state0 = np.zeros(WIN, dtype=np.float32)
    state0[-1] = 1.0

    nc = _get_neff(K_HAPPY, True, dvec)
    res = _run(nc, _feeds(inputs, a, wvec, dvec, state0, 0, K_HAPPY, True))
    r0 = res.results[0]
    out = np.array(r0["out"], dtype=np.float32)
    state = np.array(r0["st_out"], dtype=np.float32)

    if np.isnan(state).all():
        # Certified: every later step is NaN irrespective of the rates.
        return out

    # Generic fallback: continue the exact recurrence chunk by chunk.
    nc_c = _get_neff(K_CHUNK, False, dvec)
    t = K_HAPPY
    while t < T_FULL:
        k = min(K_CHUNK, T_FULL - t)
        xc = inputs[t : t + k]
        if k < K_CHUNK:  # pad (rates of padded rows can't affect emitted outputs)
            xc = np.concatenate([xc, np.zeros((K_CHUNK - k, NW), np.float32)], axis=0)
        rc = _run(nc_c, _feeds(xc, a, wvec, dvec, state, 0, K_CHUNK, False)).results[0]
        out[t : t + k] = np.array(rc["out"], dtype=np.float32)[:k]
        state = np.array(rc["st_out"], dtype=np.float32)
        t += k
        if np.isnan(state).all() and t < T_FULL:
            out[t:] = np.nan
            break
    return out


# revision 33
# speedup vs baseline: 1.0094x; 1.0094x over previous
"""Trainium2 Bass kernel for nn_Covid19InfectionsPredictModel.

Model: rate = relu(inputs @ a)  [T];  a strictly-sequential 20-tap linear
recurrence s_t = rate_t * dot(s_{t-20..t-1}, rev_head) seeded by a unit
impulse, and the observation FIR out_t = dot(s_{t-20..t-1}, diff).

Key structural fact (pure IEEE-754 float32, no approximation): the weight
rev_head[0] is exactly 0 and diff[j] > 0 for all j. The recurrence explodes
(growth ~14x/step for the given parameter scale), hits +inf, and then
0*inf => NaN poisons the window. Once the 20-value state window is ALL-NaN
at some step t*, every later s_t and out_t is NaN *regardless of rate*
(NaN*w + ... = NaN, and rate*NaN = NaN even for rate == 0). So the exact
full output is determined by the first ~60 steps plus a runtime-verified
all-NaN certificate.

Strategy:
  1. One small NEFF computes the first K=96 steps exactly on-device:
     rate via a TensorE matvec (the host passes the 96-row input slice
     pre-transposed, with `a` appended as one extra column - pure layout
     changes), per-step pre-scaled weights W[t] = rate_t * rev_head as a
     TensorE outer product flattened t-major by an SBUF->SBUF DMA, the
     sequential recurrence as ONE fused DVE scalar_tensor_tensor
     (accum_out) per step, the output FIR as TensorE matvecs over
     shifted windows whose chains fully overlap the sequential loop
     (last 19 outputs as per-output fused accum ops on DVE), plus the
     NaN tail out[96:] as a single DRAM->DRAM broadcast DMA and the
     final 20-value state window.
  2. Host checks the returned state window. All-NaN (the certified,
     overwhelmingly common case) => done. Otherwise a fallback chunk
     NEFF (1024 steps per launch, same math) is compiled lazily and
     looped over the remaining sequence, so the kernel is exactly
     correct for ANY inputs, just slower in the never-taken branch.

Measured on HW: ~36.5us NEFF exec time, of which ~12.6us is the fixed
NEFF preamble/postamble floor (measured with a trivial 2-DMA NEFF) and
~21us is the DVE sequential chain (96 dependent fused ops, ~220ns
dependent-op turnaround each).

The recurrence is strictly sequential and tiny per step; there is no
batch/scenario dimension in this problem instance, so the 8 cores run the
program SPMD-replicated (core 0's output is gathered) - intra-sequence
sharding has nothing to win (see sharding hint). The sequential step is
latency-bound at one small DVE op (~165ns); step-pairing rewrites were
rejected because they provably shift the inf->NaN onset by one position.
"""

from contextlib import ExitStack

import numpy as np

T_FULL = 65536
NW = 64
L = 21
WIN = 20  # recurrence window (L-1 taps)
K_HAPPY = 96
K_CHUNK = 1024
NAN_COLS = 2045  # (T_FULL - K_HAPPY) == 32 * 2045

_cache = {}


def _weights_from_h(h):
    """rev_head (wvec) and diff (dvec) exactly as the reference computes them."""
    h0 = np.float32(np.asarray(h).reshape(-1)[0])
    t = np.arange(L, dtype=np.float32)
    surv = np.exp(-t / h0).astype(np.float32)
    surv = ((surv - surv[-1]) / (np.float32(1.0) - surv[-1])).astype(np.float32)
    rev = surv[::-1].copy()  # reverse_surv, rev[20] == 1, rev[0] == 0
    wvec = rev[:WIN].copy()  # rev_head, wvec[0] == 0 exactly
    dvec = (rev[1:] - rev[:-1]).copy()  # diff, all > 0
    return wvec, dvec


def _build(K, full_out, dvec):
    """Emit + compile the K-step recurrence NEFF. Returns the Bacc object."""
    import concourse.bacc as bacc
    import concourse.bass as bass
    import concourse.tile as tile
    from concourse import mybir

    f32 = mybir.dt.float32
    mult = mybir.AluOpType.mult

    nc = bacc.Bacc("TRN2", num_devices=1, debug=False)
    # xTa = [inputs[t0:t0+K].T | a] : one DMA covers both matmul operands.
    xTa = nc.dram_tensor("xTa", [NW, K + 1], f32, kind="ExternalInput").ap()
    # aux = [wvec(20) | dvec(20) | state0(20)] : one DMA lands the FIR taps,
    # wvec, and the s-buffer seed.
    aux = nc.dram_tensor("aux", [3 * WIN], f32, kind="ExternalInput").ap()
    dv = nc.dram_tensor("dv", [WIN], f32, kind="ExternalInput").ap()
    if full_out:
        nansrc = nc.dram_tensor("nansrc", [NAN_COLS], f32, kind="ExternalInput").ap()
        out = nc.dram_tensor("out", [T_FULL], f32, kind="ExternalOutput").ap()
    else:
        out = nc.dram_tensor("out", [K], f32, kind="ExternalOutput").ap()
    st_out = nc.dram_tensor("st_out", [WIN], f32, kind="ExternalOutput").ap()

    with tile.TileContext(nc) as tc, ExitStack() as ctx:
        pool = ctx.enter_context(tc.tile_pool(name="p", bufs=1))
        psp = ctx.enter_context(tc.tile_pool(name="ps", bufs=1, space="PSUM"))

        xTa_sb = pool.tile([NW, K + 1], f32)
        nc.sync.dma_start(out=xTa_sb, in_=xTa)
        # aux_sb[0, 0:20]=wvec, [20:40]=dvec, [40:60]=state0 = s_buf[0:20].
        aux_sb = pool.tile([1, 2 * WIN + K + WIN], f32)
        nc.sync.dma_start(out=aux_sb[:, : 3 * WIN], in_=aux.rearrange("(o k) -> o k", o=1))
        w_row = aux_sb[:, 0:WIN]
        dv_row = aux_sb[:, WIN : 2 * WIN]
        s_buf = aux_sb[:, 2 * WIN : 2 * WIN + K + WIN]
        d_col = pool.tile([WIN, 1], f32)
        nc.gpsimd.dma_start(out=d_col, in_=dv.rearrange("(k o) -> k o", o=1))

        # rate_row[0, t] = relu(inputs[t] @ a), flat on partition 0 (relu as a
        # DVE max against 0.0 straight out of PSUM - no ScalarE on this path).
        rate_row = pool.tile([1, K], f32)
        for m in range(0, K, 512):
            n = min(512, K - m)
            r_ps = psp.tile([1, 512], f32, tag="r_ps")
            nc.tensor.matmul(
                r_ps[:, :n],
                lhsT=xTa_sb[:, K : K + 1],
                rhs=xTa_sb[:, m : m + n],
                start=True,
                stop=True,
            )
            nc.vector.tensor_scalar_max(rate_row[:, m : m + n], r_ps[:, :n], 0.0)

        # Pre-scaled per-step weights W[t, j] = rate_t * wvec[j]: TensorE outer
        # product (blocked to the 128-partition PSUM limit), flattened t-major
        # by one SBUF->SBUF DMA per block so each step reads a contiguous
        # 20-element slice on partition 0.
        wf = pool.tile([1, K * WIN], f32)
        for c in range(0, K, 128):
            blk = min(128, K - c)
            w_ps = psp.tile([128, WIN], f32, tag="w_ps")
            nc.tensor.matmul(
                w_ps[:blk], lhsT=rate_row[:, c : c + blk], rhs=w_row, start=True, stop=True
            )
            w2d = pool.tile([128, WIN], f32, tag="w2d")
            nc.scalar.activation(w2d[:blk], w_ps[:blk], mybir.ActivationFunctionType.Copy)
            nc.sync.dma_start(out=wf[:, c * WIN : (c + blk) * WIN], in_=w2d[:blk])

        # The sequential chain: one fused multiply+accumulate-reduce per step.
        # First S0 steps read rate_t as a pointer scalar (available ~2.5us
        # before the flattened wf buffer), computing (wvec*rate_t)*s — bitwise
        # identical to the wf form fl(fl(w*r)*s) — so the chain starts as soon
        # as the rate row exists; the rest use the cheaper immediate form.
        S0 = min(16, K)
        junk = pool.tile([1, WIN], f32)
        for t in range(S0):
            nc.vector.scalar_tensor_tensor(
                out=junk,
                in0=w_row,
                scalar=rate_row[:, t : t + 1],
                in1=s_buf[:, t : t + WIN],
                op0=mult,
                op1=mult,
                accum_out=s_buf[:, WIN + t : WIN + t + 1],
            )
        for t in range(S0, K):
            o = t * WIN
            nc.vector.scalar_tensor_tensor(
                out=junk,
                in0=s_buf[:, t : t + WIN],
                scalar=1.0,
                in1=wf[:, o : o + WIN],
                op0=mult,
                op1=mult,
                accum_out=s_buf[:, WIN + t : WIN + t + 1],
            )

        nc.gpsimd.dma_start(
            out=st_out.rearrange("(o k) -> o k", o=1), in_=s_buf[:, K : K + WIN]
        )

        # Observation FIR out[t] = sum_j s[t+j] * diff[j]: two TensorE matvec
        # chunks over 20 shifted copies of s. Each chunk needs its last s value
        # at least TAILN=19 steps before the seq loop ends, so both chunk
        # chains (DMA -> PE -> ScalarE copy -> DMA out) fully overlap the DVE
        # sequential loop; only the last TAILN outputs run on DVE after it.
        TAILN = 19
        kpe = K - TAILN  # 77
        for f0, ln in ((0, (kpe + 1) // 2), ((kpe + 1) // 2, kpe - (kpe + 1) // 2)):
            # Bounce this chunk's s-slice through DRAM: the SBUF-side write
            # uses a plain range-tracked AP (so it waits for exactly the seq
            # steps it needs), and the overlapping-window read is from DRAM
            # where whole-tensor dependency tracking orders it correctly.
            sscr = nc.dram_tensor(f"sscr{f0}", [ln + WIN - 1], f32, kind="Internal").ap()
            nc.sync.dma_start(
                out=sscr.rearrange("(o k) -> o k", o=1),
                in_=s_buf[:, f0 : f0 + ln + WIN - 1],
            )
            s_sh = pool.tile([WIN, ln], f32, name=f"ssh{f0}")
            src = bass.AP(tensor=sscr.tensor, offset=sscr.offset, ap=[[1, WIN], [1, ln]])
            nc.sync.dma_start(out=s_sh, in_=src)
            o_ps = psp.tile([1, 512], f32, tag=f"ops{f0}")
            nc.tensor.matmul(o_ps[:, :ln], lhsT=d_col, rhs=s_sh, start=True, stop=True)
            o_sb = pool.tile([1, ln], f32, name=f"osb{f0}")
            nc.scalar.activation(o_sb, o_ps[:, :ln], mybir.ActivationFunctionType.Copy)
            nc.sync.dma_start(
                out=out[f0 : f0 + ln].rearrange("(o k) -> o k", o=1), in_=o_sb
            )

        out_c = pool.tile([1, TAILN], f32)
        junk2 = pool.tile([1, WIN], f32)
        for i in range(TAILN):
            t = kpe + i
            nc.vector.scalar_tensor_tensor(
                out=junk2,
                in0=dv_row,
                scalar=1.0,
                in1=s_buf[:, t : t + WIN],
                op0=mult,
                op1=mult,
                accum_out=out_c[:, i : i + 1],
            )
        nc.sync.dma_start(
            out=out[kpe:K].rearrange("(o k) -> o k", o=1), in_=out_c
        )

        if full_out:
            # NaN tail: one DRAM->DRAM broadcast DMA on the GPSIMD queues,
            # no engine time and no Sync-sequencer issue slot.
            tail_dst = out[K:T_FULL].rearrange("(r c) -> r c", c=NAN_COLS)
            tail_src = bass.AP(
                tensor=nansrc.tensor, offset=nansrc.offset, ap=[[0, 32], [1, NAN_COLS]]
            )
            nc.gpsimd.dma_start(out=tail_dst, in_=tail_src)

    nc.compile()
    return nc


def _get_neff(K, full_out, dvec):
    key = (K, full_out, dvec.tobytes())
    if key not in _cache:
        _cache[key] = _build(K, full_out, dvec)
    return _cache[key]


def _run(nc, feeds, trace=False):
    from concourse import bass_utils

    in_maps = [dict(feeds) for _ in range(8)]
    res = bass_utils.run_bass_kernel_spmd(nc, in_maps, core_ids=list(range(8)), trace=trace)
    return res


def _feeds(inputs, a, wvec, dvec, state, t0, K, full_out):
    xTa = np.empty((NW, K + 1), dtype=np.float32)
    xTa[:, :K] = inputs[t0 : t0 + K].T
    xTa[:, K] = a
    aux = np.concatenate([wvec, dvec, state]).astype(np.float32)
    f = {"xTa": xTa, "aux": aux, "dv": dvec}
    if full_out:
        f["nansrc"] = np.full(NAN_COLS, np.nan, dtype=np.float32)
    return f


def kernel(inputs, a, h):
    inputs = np.ascontiguousarray(np.asarray(inputs, dtype=np.float32))
    a = np.ascontiguousarray(np.asarray(a, dtype=np.float32))
    wvec, dvec = _weights_from_h(h)
    state0 = np.zeros(WIN, dtype=np.float32)
    state0[-1] = 1.0

    nc = _get_neff(K_HAPPY, True, dvec)
    res = _run(nc, _feeds(inputs, a, wvec, dvec, state0, 0, K_HAPPY, True))
    r0 = res.results[0]
    out = np.array(r0["out"], dtype=np.float32)
    state = np.array(r0["st_out"], dtype=np.float32)

    if np.isnan(state).all():
        # Certified: every later step is NaN irrespective of the rates.
        return out

    # Generic fallback: continue the exact recurrence chunk by chunk.
    nc_c = _get_neff(K_CHUNK, False, dvec)
    t = K_HAPPY
    while t < T_FULL:
        k = min(K_CHUNK, T_FULL - t)
        xc = inputs[t : t + k]
        if k < K_CHUNK:  # pad (rates of padded rows can't affect emitted outputs)
            xc = np.concatenate([xc, np.zeros((K_CHUNK - k, NW), np.float32)], axis=0)
        rc = _run(nc_c, _feeds(xc, a, wvec, dvec, state, 0, K_CHUNK, False)).results[0]
        out[t : t + k] = np.array(rc["out"], dtype=np.float32)[:k]
        state = np.array(rc["st_out"], dtype=np.float32)
        t += k
        if np.isnan(state).all() and t < T_FULL:
            out[t:] = np.nan
            break
    return out
